# revision 1
# baseline (speedup 1.0000x reference)
"""Trainium2 Bass kernel for nn_Expert (gather-span + 2-layer linear MLP).

Reference computation (B=32, L=4096, H=1024, N=4):
    idx      = pos + arange(N)                      # (B, N)
    gathered = hidden[b, idx[b, n], :]              # (B, N, H)
    x        = gathered.reshape(B, N*H)             # (B, 4096)
    out      = (x @ W1.T + b1) @ W2.T + b2          # (B, 4)

Sharding (8 cores): hidden is sharded on the LAST dim (H) in 128-wide
slices; W1 is sharded over the matching contraction columns (a 2MB read
per core instead of a replicated 16MB one -- W1 is the dominant HBM
traffic and the problem is memory-bound); pos and W2 are replicated;
biases ride with core 0 only (zeros elsewhere). Per core:
  1. pos arrives as one contiguous (1, 128) row (replicated 4x n-major)
     and is PE-transposed onto partitions; the static part of the gather
     index (b*L + n for partition p = n*32+b) is built with 4 iotas; one
     int add forms idx[p] = b*L + pos[b] + n,
  2. indirect-DMA gather of the 128 span-rows -> xg (128, 128),
  3. one 128x128 PE transpose -> xT (contraction dim on partitions),
  4. stage 1 on PE with x stationary, W1 streaming from 4 pipelined
     512KB tiles into two (32, 512) PSUM accumulators,
  5. out1 (+b1 on core 0) is replicated to all 4 partition quadrants
     with an extra ones-column so stage 2 picks up b2 from W2's padding,
  6. stage 2 on DVE at full 128-partition occupancy:
     y[t*32+b] = sum_o rep[p, o] * w2p[p, o],
  7. y is PE-transposed to (1, 128) so the output DMA is one contiguous
     descriptor; the host sums per-core partials and reshapes to (B, N).
All DMAs move >=512B-contiguous chunks (per-partition 4KB for W1) --
per-4B-packet DMA patterns cost ~25-50ns/packet on this part.
The contraction split (4096 = 8 cores x 4 chunks x 128) only
reassociates fp32 sums the way any tiled matmul does.
"""

import numpy as np

from concourse import bass, bacc, mybir
from concourse.tile import TileContext
from concourse.bass_utils import run_bass_kernel_spmd
from concourse.masks import make_identity

B, L, H, N = 32, 4096, 1024, 4
NCORES = 8
HS = H // NCORES       # 128: per-core slice of the hidden dim
P = 128
HB = H // 2            # 512: psum bank width for stage 1
F32 = mybir.dt.float32
I32 = mybir.dt.int32

TRACE = False          # set True in test harnesses to profile
LAST_EXEC_NS = None

_nc_cache = None


def _build_nc():
    nc = bacc.Bacc(target_bir_lowering=False)
    hid = nc.declare_dram_parameter("hid", [B * L, HS], F32, isOutput=False)
    posf = nc.declare_dram_parameter("posf", [1, P], F32, isOutput=False)
    w1t = nc.declare_dram_parameter("w1t", [N * P, H], F32, isOutput=False)
    w2p = nc.declare_dram_parameter("w2p", [P, H + 1], F32, isOutput=False)
    b1r = nc.declare_dram_parameter("b1r", [B, H], F32, isOutput=False)
    out = nc.declare_dram_parameter("out", [1, P], F32, isOutput=True)

    with TileContext(nc) as tc:
        with (
            tc.tile_pool(name="sbuf", bufs=1) as spool,
            tc.tile_pool(name="ps1", bufs=2, space="PSUM") as ppool,
            tc.tile_pool(name="psx", bufs=1, space="PSUM") as xpool,
        ):
            # ---- gather-index chain (no partition-strided DMAs anywhere)
            posf_sb = spool.tile([1, P], F32)
            nc.sync.dma_start(out=posf_sb[:], in_=posf[:])

            ident = spool.tile([P, P], F32)
            make_identity(nc, ident[:])

            posT_ps = xpool.tile([P, 1], F32, space="PSUM", tag="post")
            nc.tensor.transpose(
                out=posT_ps[:], in_=posf_sb[:], identity=ident[:1, :1]
            )
            posi = spool.tile([P, 1], I32)
            nc.vector.tensor_copy(out=posi[:], in_=posT_ps[:])

            gc = spool.tile([P, 1], I32)
            for q in range(N):
                # slice-relative iota: gc[q*32+b] = q + b*L
                nc.gpsimd.iota(
                    gc[q * B:(q + 1) * B, :], pattern=[[0, 1]], base=q,
                    channel_multiplier=L,
                )
            idx = spool.tile([P, 1], I32)
            nc.gpsimd.tensor_tensor(
                out=idx[:], in0=gc[:], in1=posi[:], op=mybir.AluOpType.add
            )

            xg = spool.tile([P, HS], F32)
            nc.gpsimd.indirect_dma_start(
                out=xg[:, :],
                out_offset=None,
                in_=hid[:],
                in_offset=bass.IndirectOffsetOnAxis(ap=idx[:, :1], axis=0),
                bounds_check=B * L - 1,
                oob_is_err=False,
            )

            # ---- W1 streams in 4 pipelined tiles on the SP queues
            w1sb = []
            for n in range(N):
                t = spool.tile([P, H], F32, tag=f"w1_{n}", name=f"w1_{n}")
                nc.sync.dma_start(out=t[:], in_=w1t[n * P:(n + 1) * P, :])
                w1sb.append(t)
            # stage-2 operands on ACT (not needed until late)
            w2sb = spool.tile([P, H + 1], F32)
            nc.scalar.dma_start(out=w2sb[:], in_=w2p[:])
            b1sb = spool.tile([B, H], F32)
            nc.scalar.dma_start(out=b1sb[:], in_=b1r[:])

            # ---- transpose: xT[k, p] = xg[p, k]
            xT_ps = xpool.tile([P, P], F32, space="PSUM", tag="xt")
            nc.tensor.transpose(out=xT_ps[:], in_=xg[:], identity=ident[:])
            xT = spool.tile([P, P], F32)
            nc.vector.tensor_copy(out=xT[:], in_=xT_ps[:])

            # ---- stage 1: out1[b, o] = sum_{n,k} x[b, nk] W1[o, nk]
            ps = [
                ppool.tile([B, HB], F32, space="PSUM", tag="ps1",
                           name=f"ps1_{i}")
                for i in range(2)
            ]
            for n in range(N):
                for half in range(2):
                    nc.tensor.matmul(
                        out=ps[half][:],
                        lhsT=xT[:, n * B:(n + 1) * B],
                        rhs=w1sb[n][:, half * HB:(half + 1) * HB],
                        start=(n == 0),
                        stop=(n == N - 1),
                    )

            # ---- replicate out1 (+b1, +ones col) to all 4 quadrants
            rep = spool.tile([P, H + 1], F32)
            for half in range(2):
                nc.vector.tensor_tensor(
                    out=rep[:B, half * HB:(half + 1) * HB],
                    in0=ps[half][:],
                    in1=b1sb[:, half * HB:(half + 1) * HB],
                    op=mybir.AluOpType.add,
                )
            nc.vector.memset(rep[:B, H:H + 1], 1.0)
            for q in range(1, 4):
                nc.sync.dma_start(
                    out=rep[q * B:(q + 1) * B, :], in_=rep[:B, :]
                )

            # ---- stage 2 (DVE, full 128-partition occupancy)
            prod = spool.tile([P, H + 1], F32)
            nc.vector.tensor_tensor(
                out=prod[:], in0=rep[:], in1=w2sb[:], op=mybir.AluOpType.mult
            )
            y128 = spool.tile([P, 1], F32)
            nc.vector.tensor_reduce(
                out=y128[:], in_=prod[:], op=mybir.AluOpType.add,
                axis=mybir.AxisListType.X,
            )

            # ---- transpose y to one row so the output DMA is contiguous
            yT_ps = xpool.tile([1, P], F32, space="PSUM", tag="yt")
            nc.tensor.transpose(out=yT_ps[:], in_=y128[:], identity=ident[:])
            yT = spool.tile([1, P], F32)
            nc.vector.tensor_copy(out=yT[:], in_=yT_ps[:])
            nc.sync.dma_start(out=out[:], in_=yT[:])

    nc.finalize()
    return nc


def _get_nc():
    global _nc_cache
    if _nc_cache is None:
        _nc_cache = _build_nc()
    return _nc_cache


def kernel(hidden, pos, W1, b1, W2, b2):
    global LAST_EXEC_NS
    hidden = np.asarray(hidden, dtype=np.float32)
    pos = np.asarray(pos)
    W1 = np.asarray(W1, dtype=np.float32)
    b1 = np.asarray(b1, dtype=np.float32)
    W2 = np.asarray(W2, dtype=np.float32)
    b2 = np.asarray(b2, dtype=np.float32)

    # pos as one contiguous f32 row, replicated n-major: posf[n*32+b]=pos[b]
    posf = np.tile(pos.reshape(B).astype(np.float32), N)[None, :]

    # W1 (H, N*H) -> per-core (N*P, H): w1t_j[n*P+k, o] = W1[o, n*H+j*HS+k]
    w1r = W1.reshape(H, N, NCORES, HS)                 # [o, n, j, k]
    # W2 replicated by quadrant, ones-column carries b2 (core 0 only)
    w2p0 = np.concatenate(
        [np.repeat(W2, B, axis=0), np.repeat(b2, B)[:, None]], axis=1
    ).astype(np.float32)                               # (128, 1025)
    w2pz = np.concatenate(
        [np.repeat(W2, B, axis=0), np.zeros((P, 1), np.float32)], axis=1
    ).astype(np.float32)
    b1r0 = np.ascontiguousarray(np.broadcast_to(b1, (B, H)))
    b1rz = np.zeros((B, H), np.float32)

    in_maps = []
    for j in range(NCORES):
        hid_j = np.ascontiguousarray(
            hidden[:, :, j * HS:(j + 1) * HS]
        ).reshape(B * L, HS)
        w1t_j = np.ascontiguousarray(
            w1r[:, :, j, :].transpose(1, 2, 0).reshape(N * P, H)
        )
        in_maps.append(
            {
                "hid": hid_j,
                "posf": posf,
                "w1t": w1t_j,
                "w2p": w2p0 if j == 0 else w2pz,
                "b1r": b1r0 if j == 0 else b1rz,
            }
        )

    nc = _get_nc()
    res = run_bass_kernel_spmd(nc, in_maps, list(range(NCORES)), trace=TRACE)
    LAST_EXEC_NS = res.exec_time_ns

    parts = np.stack([res.results[j]["out"] for j in range(NCORES)])  # (8,1,128)
    y128 = parts.sum(axis=0, dtype=np.float64).reshape(N, B)          # [t, b]
    return np.ascontiguousarray(y128.T.astype(np.float32))            # (B, N)



# revision 6
# speedup vs baseline: 1.1422x; 1.1422x over previous
"""Trainium2 Bass kernel for nn_Expert (gather-span + 2-layer linear MLP).

Reference computation (B=32, L=4096, H=1024, N=4):
    idx      = pos + arange(N)                      # (B, N)
    gathered = hidden[b, idx[b, n], :]              # (B, N, H)
    x        = gathered.reshape(B, N*H)             # (B, 4096)
    out      = (x @ W1.T + b1) @ W2.T + b2          # (B, 4)

Sharding (8 cores): the contraction dim (N*H = 4096) is split across
cores -- hidden sharded on its last dim in 128-wide slices, W1 sharded
over the matching contraction columns.  Each core reads 2 MB of W1 (the
dominant, irreducible HBM traffic; the problem is memory-bound) plus a
64 KB gathered slice of hidden, computes partial out1/(N,B) y, and the
host sums the 8 partials.

Per-core structure, ordered so the W1 stream is the only binding cost:
  1. W1 streams as 8x 256 KB DMAs (column-halves of 4 contraction
     tiles) split across the two HWDGE rings (sync + scalar).  Fine
     granularity lets the SDMA engines' round-robin slot the gather in
     early, and lets each matmul start on exactly the bytes it needs.
  2. In parallel: row indices b*L+pos[b]+n (host-precomputed layout
     arithmetic) land on 128 partitions, one indirect DMA gathers the
     128 span-rows -> xg (128,128), one PE transpose -> xT.
  3. b1 enters PSUM via K=1 matmuls (accumulation-group start) -- no
     bias broadcast traffic, no separate add.
  4. Stage 1 is column-tiled on the PE: group A (psum partitions 0-31)
     computes out1[:, 0:512], group B (partitions 32-63, col_grp q32)
     computes out1[:, 512:1024].  The two groups' fp32 passes run
     concurrently in disjoint 32-col strips of the systolic array,
     halving the (HAM-cold) PE serial time so it stays under the DMA.
  5. Stage 2: out1 -> SBUF (vector+scalar in parallel), 8 PE
     transposes put each 128-chunk of the hidden dim on partitions,
     8 small matmuls against W2 slices accumulate y (4, 32) in PSUM;
     b2 rides a (4,32) pre-broadcast add on the copy out.
The host sums per-core y partials (fp64) and transposes to (B, N).
"""

import numpy as np

from concourse import bass, bacc, mybir
from concourse.tile import TileContext
from concourse.bass_utils import run_bass_kernel_spmd
from concourse.masks import make_identity

B, L, H, N = 32, 4096, 1024, 4
NCORES = 8
HS = H // NCORES       # 128: per-core slice of the hidden dim
P = 128
HB = H // 2            # 512: psum bank width for stage 1
F32 = mybir.dt.float32
I32 = mybir.dt.int32

TRACE = False          # set True in test harnesses to profile
LAST_EXEC_NS = None

_nc_cache = None


def _build_nc():
    nc = bacc.Bacc(target_bir_lowering=False)
    hid = nc.declare_dram_parameter("hid", [B * L, HS], F32, isOutput=False)
    idxd = nc.declare_dram_parameter("idxd", [1, P], I32, isOutput=False)
    w1t = nc.declare_dram_parameter("w1t", [N * P, H], F32, isOutput=False)
    w2d = nc.declare_dram_parameter("w2d", [P, 2 * N * N], F32, isOutput=False)
    b1d = nc.declare_dram_parameter("b1d", [1, H], F32, isOutput=False)
    b2d = nc.declare_dram_parameter("b2d", [N, B], F32, isOutput=False)
    outd = nc.declare_dram_parameter("out", [N, B], F32, isOutput=True)

    with TileContext(nc) as tc:
        with (
            tc.tile_pool(name="sbuf", bufs=1) as spool,
            tc.tile_pool(name="ps1", bufs=1, space="PSUM") as pool1,
            tc.tile_pool(name="psX", bufs=1, space="PSUM") as poolX,
            tc.tile_pool(name="psT", bufs=1, space="PSUM") as poolT,
            tc.tile_pool(name="psY", bufs=1, space="PSUM") as poolY,
        ):
            # ---- DMA triggers, most-urgent first on each HWDGE ring.
            # W1 pieces: (tile n, column half h) -> w1sb[n][:, 512h:512(h+1)]
            w1sb = [
                spool.tile([P, H], F32, tag=f"w1_{n}", name=f"w1_{n}")
                for n in range(N)
            ]

            def w1piece(n, h):
                nc_eng = nc.sync if n in (0, 1) else nc.scalar
                nc_eng.dma_start(
                    out=w1sb[n][:, h * HB:(h + 1) * HB],
                    in_=w1t[n * P:(n + 1) * P, h * HB:(h + 1) * HB],
                )

            # sync ring: gather indices first, then W1 tiles 0/1
            idxi = spool.tile([P, 1], I32)
            nc.sync.dma_start(out=idxi[:], in_=idxd[:])
            # scalar ring: b1 first (unlocks psum-init matmuls)
            b1sb = spool.tile([1, H], F32)
            nc.scalar.dma_start(out=b1sb[:], in_=b1d[:])
            for h in range(2):          # halves in A-then-B order per ring
                w1piece(0, h)
                w1piece(2, h)
                w1piece(1, h)
                w1piece(3, h)
            w2sb = spool.tile([P, 2 * N * N], F32)
            nc.scalar.dma_start(out=w2sb[:], in_=w2d[:])
            b2sb = spool.tile([N, B], F32)
            nc.scalar.dma_start(out=b2sb[:], in_=b2d[:])

            # ---- identity for PE transposes; ones row for the b1 matmul
            ident = spool.tile([P, P], F32)
            make_identity(nc, ident[:])
            ones1 = spool.tile([1, B], F32)
            nc.vector.memset(ones1[:], 1.0)

            # ---- gather: one indirect DMA of the 128 span-rows
            xg = spool.tile([P, HS], F32)
            nc.gpsimd.indirect_dma_start(
                out=xg[:, :],
                out_offset=None,
                in_=hid[:],
                in_offset=bass.IndirectOffsetOnAxis(ap=idxi[:, :1], axis=0),
                bounds_check=B * L - 1,
                oob_is_err=False,
            )

            # ---- stage 1, column-tiled: A = psum rows 0:32 (out1 cols
            # 0:512), B = psum rows 32:64 (out1 cols 512:1024).
            psAB = pool1.tile([2 * B, HB], F32, space="PSUM", tag="ps_ab")

            def mm1(grp, lhsT, rhs, start, stop):
                nc.tensor.matmul(
                    out=psAB[grp * B:(grp + 1) * B, :],
                    lhsT=lhsT, rhs=rhs,
                    start=start, stop=stop, skip_group_check=True,
                )

            mm1(0, ones1[:], b1sb[:1, 0:HB], True, False)
            mm1(1, ones1[:], b1sb[:1, HB:H], True, False)

            # transpose the gather: xT[k, n*32+b] = xg[n*32+b, k]
            xtp = poolX.tile([P, P], F32, space="PSUM", tag="xtp")
            nc.tensor.transpose(out=xtp[:], in_=xg[:], identity=ident[:])
            xT = spool.tile([P, P], F32)
            nc.vector.tensor_copy(out=xT[:], in_=xtp[:])

            # matmuls ordered by expected piece arrival on the two rings
            for n, grp, stop in (
                (0, 0, False), (2, 0, False), (0, 1, False), (2, 1, False),
                (1, 0, False), (3, 0, True), (1, 1, False), (3, 1, True),
            ):
                mm1(
                    grp,
                    xT[:, n * B:(n + 1) * B],
                    w1sb[n][:, grp * HB:(grp + 1) * HB],
                    False, stop,
                )

            # ---- stage 2: y[t, b] = b2[t] + sum_o W2[t, o] out1[b, o]
            o1sb = spool.tile([2 * B, HB], F32)
            nc.vector.tensor_copy(out=o1sb[0:B, :], in_=psAB[0:B, :])
            nc.scalar.copy(out=o1sb[B:2 * B, :], in_=psAB[B:2 * B, :])

            trp = poolT.tile([P, 2 * P], F32, space="PSUM", tag="trp")
            trsb = spool.tile([P, 2 * P], F32)
            yps = poolY.tile([N, B], F32, space="PSUM", tag="yps")
            NC = H // P  # 8 chunks of the hidden dim; 0-3 from A, 4-7 from B
            for c in range(NC):
                g = c // 4
                nc.tensor.transpose(
                    out=trp[:, c * B:(c + 1) * B],
                    in_=o1sb[g * B:(g + 1) * B, (c % 4) * P:(c % 4 + 1) * P],
                    identity=ident[g * B:(g + 1) * B, g * B:(g + 1) * B],
                )
                cp = nc.vector.tensor_copy if c % 2 == 0 else (
                    lambda out, in_: nc.scalar.copy(out=out, in_=in_)
                )
                cp(
                    out=trsb[:, c * B:(c + 1) * B],
                    in_=trp[:, c * B:(c + 1) * B],
                )
                nc.tensor.matmul(
                    out=yps[:],
                    lhsT=w2sb[:, c * N:(c + 1) * N],
                    rhs=trsb[:, c * B:(c + 1) * B],
                    start=(c == 0), stop=(c == NC - 1),
                )

            ysb = spool.tile([N, B], F32)
            nc.vector.tensor_tensor(
                out=ysb[:], in0=yps[:], in1=b2sb[:], op=mybir.AluOpType.add
            )
            nc.sync.dma_start(out=outd[:], in_=ysb[:])

    nc.finalize()
    return nc


def _get_nc():
    global _nc_cache
    if _nc_cache is None:
        _nc_cache = _build_nc()
    return _nc_cache


def kernel(hidden, pos, W1, b1, W2, b2):
    global LAST_EXEC_NS
    hidden = np.asarray(hidden, dtype=np.float32)
    pos = np.asarray(pos)
    W1 = np.asarray(W1, dtype=np.float32)
    b1 = np.asarray(b1, dtype=np.float32)
    W2 = np.asarray(W2, dtype=np.float32)
    b2 = np.asarray(b2, dtype=np.float32)

    # gather row indices in the per-core (B*L, HS) layout, n-major on
    # partitions: idxd[n*32+b] = b*L + pos[b] + n
    pvec = pos.reshape(B).astype(np.int64)
    idxd = (
        np.tile(np.arange(B, dtype=np.int64) * L + pvec, N)
        + np.repeat(np.arange(N, dtype=np.int64), B)
    ).astype(np.int32)[None, :]

    # W1 (H, N*H) -> per-core (N*P, H): w1t_j[n*P+k, o] = W1[o, n*H+j*HS+k]
    w1r = W1.reshape(H, N, NCORES, HS)                 # [o, n, j, k]
    # W2 (N, H) -> (128, 32): w2d[k2, c*N+t] = W2[t, c*P+k2]
    w2d = np.ascontiguousarray(
        W2.reshape(N, H // P, P).transpose(2, 1, 0).reshape(P, -1)
    )
    b1r = b1[None, :]                                  # (1, H), core 0 only
    b1z = np.zeros((1, H), np.float32)
    b2r = np.ascontiguousarray(
        np.broadcast_to(b2[:, None], (N, B))
    )                                                  # (4, 32), core 0 only
    b2z = np.zeros((N, B), np.float32)

    in_maps = []
    for j in range(NCORES):
        hid_j = np.ascontiguousarray(
            hidden[:, :, j * HS:(j + 1) * HS]
        ).reshape(B * L, HS)
        w1t_j = np.ascontiguousarray(
            w1r[:, :, j, :].transpose(1, 2, 0).reshape(N * P, H)
        )
        in_maps.append(
            {
                "hid": hid_j,
                "idxd": idxd,
                "w1t": w1t_j,
                "w2d": w2d,
                "b1d": b1r if j == 0 else b1z,
                "b2d": b2r if j == 0 else b2z,
            }
        )

    nc = _get_nc()
    res = run_bass_kernel_spmd(nc, in_maps, list(range(NCORES)), trace=TRACE)
    LAST_EXEC_NS = res.exec_time_ns

    parts = np.stack([res.results[j]["out"] for j in range(NCORES)])  # (8,4,32)
    ytb = parts.sum(axis=0, dtype=np.float64)                         # (4, 32)
    return np.ascontiguousarray(ytb.T.astype(np.float32))             # (B, N)


# revision 10
# speedup vs baseline: 1.1453x; 1.0027x over previous
"""Trainium2 Bass kernel for nn_Expert (gather-span + 2-layer linear MLP).

Reference computation (B=32, L=4096, H=1024, N=4):
    idx      = pos + arange(N)                      # (B, N)
    gathered = hidden[b, idx[b, n], :]              # (B, N, H)
    x        = gathered.reshape(B, N*H)             # (B, 4096)
    out      = (x @ W1.T + b1) @ W2.T + b2          # (B, 4)

Sharding (8 cores): the contraction dim (N*H = 4096) is split across
cores -- hidden sharded on its last dim in 128-wide slices, W1 sharded
over the matching contraction columns.  Each core reads 2 MB of W1 (the
dominant, irreducible HBM traffic; the problem is memory-bound) plus a
64 KB gathered slice of hidden, computes partial out1/(N,B) y partials,
and the host sums the 8 partials.

Schedule notes (from profiling on this part):
  * The indirect (SWDGE) gather's packets only run when the HWDGE
    rings are idle -- they do NOT round-robin with an active W1
    stream.  So the gather goes FIRST: its descriptors are generated
    while only tiny transfers are in flight, and the sync-ring W1
    pieces are released only after descriptor generation (enforced by
    a WAW dep: tiny gpsimd memsets into the W1 tiles, ordered after
    the indirect DMA).  The scalar ring streams immediately -- the
    gather slots in after its first 256 KB batch.
  * The gather is 32x 2KB descriptors: one per batch row, exploiting
    that the N=4 span rows are contiguous in (B*L, 128) layout.
  * fp32 matmuls run LOW+HIGH passes at the HAM-cold 1.2 GHz rate
    (~430ns per 512-col pass); stage 1 is column-tiled (group A ->
    psum partitions 0:32 = out1[:, 0:512], group B -> 32:64, col_grp
    q32) so the two groups' passes run concurrently and PE stays
    under the DMA rate.
  * b1 enters PSUM via K=1 matmuls (group start) while the PE is
    otherwise idle; b2 rides a (4,32) pre-broadcast add at the end.
  * Stage 2 is straight-line batches (no PE<->DVE ping-pong): copy
    out1 to SBUF (vector || scalar), 8 PE transposes alternating row
    strips q0/q32, one copy, 8 accumulating y matmuls.
The host sums per-core y partials (fp64) and transposes to (B, N).
"""

import numpy as np

from concourse import bass, bacc, mybir
from concourse.tile import TileContext
from concourse.bass_utils import run_bass_kernel_spmd
from concourse.masks import make_identity

B, L, H, N = 32, 4096, 1024, 4
NCORES = 8
HS = H // NCORES       # 128: per-core slice of the hidden dim
P = 128
HB = H // 2            # 512: psum bank width for stage 1
F32 = mybir.dt.float32
I32 = mybir.dt.int32

TRACE = False          # set True in test harnesses to profile
LAST_EXEC_NS = None

_nc_cache = None


def _build_nc():
    nc = bacc.Bacc(target_bir_lowering=False)
    hid = nc.declare_dram_parameter("hid", [B * L, HS], F32, isOutput=False)
    idxd = nc.declare_dram_parameter("idxd", [1, B], I32, isOutput=False)
    w1t = nc.declare_dram_parameter("w1t", [N * P, H], F32, isOutput=False)
    w2d = nc.declare_dram_parameter("w2d", [P, 2 * N * N], F32, isOutput=False)
    b1d = nc.declare_dram_parameter("b1d", [1, H], F32, isOutput=False)
    b2d = nc.declare_dram_parameter("b2d", [N, B], F32, isOutput=False)
    outd = nc.declare_dram_parameter("out", [N, B], F32, isOutput=True)

    with TileContext(nc) as tc:
        with (
            tc.tile_pool(name="sbuf", bufs=1) as spool,
            tc.tile_pool(name="ps1", bufs=1, space="PSUM") as pool1,
            tc.tile_pool(name="psX", bufs=1, space="PSUM") as poolX,
            tc.tile_pool(name="psT", bufs=1, space="PSUM") as poolT,
            tc.tile_pool(name="psY", bufs=1, space="PSUM") as poolY,
        ):
            w1sb = [
                spool.tile([P, H], F32, tag=f"w1_{n}", name=f"w1_{n}")
                for n in range(N)
            ]

            def w1piece(n, h):
                eng = nc.sync if n in (0, 1) else nc.scalar
                eng.dma_start(
                    out=w1sb[n][:, h * HB:(h + 1) * HB],
                    in_=w1t[n * P:(n + 1) * P, h * HB:(h + 1) * HB],
                )

            # sync ring: gather indices first
            idxi = spool.tile([B, 1], I32)
            nc.sync.dma_start(out=idxi[:], in_=idxd[:])
            # scalar ring: b1 first (unlocks the psum-init matmuls)
            b1sb = spool.tile([1, H], F32)
            nc.scalar.dma_start(out=b1sb[:], in_=b1d[:])

            # ---- identity for PE transposes; ones row for the b1 matmul
            ident = spool.tile([P, P], F32)
            make_identity(nc, ident[:])
            ones1 = spool.tile([1, B], F32)
            nc.vector.memset(ones1[:], 1.0)

            # ---- gather: 32 indices, 4 contiguous 512B rows per index
            xg = spool.tile([B, N * HS], F32)
            nc.gpsimd.indirect_dma_start(
                out=xg[:, :],
                out_offset=None,
                in_=hid[:],
                in_offset=bass.IndirectOffsetOnAxis(ap=idxi[:, :1], axis=0),
                bounds_check=B * L - 1,
                oob_is_err=False,
            )
            # WAW release: the sync-ring W1 tile DMAs below overwrite these
            # rows, so they must wait for the memsets -- which sit after the
            # indirect_dma on the gpsimd queue.  Net effect: sync-ring W1
            # traffic starts only once the gather's descriptors are in the
            # rings (SWDGE packets starve under an active HWDGE stream).
            GATE_W1 = False
            if GATE_W1:
                nc.gpsimd.memset(w1sb[0][0:1, :], 0.0)
                nc.gpsimd.memset(w1sb[1][0:1, :], 0.0)

            # W1 pieces: scalar ring (tiles 2/3) streams immediately; sync
            # ring (tiles 0/1) is gated per the WAW above.
            for h in range(2):
                w1piece(0, h)
                w1piece(2, h)
            for h in range(2):
                w1piece(1, h)
                w1piece(3, h)
            w2sb = spool.tile([P, 2 * N * N], F32)
            nc.scalar.dma_start(out=w2sb[:], in_=w2d[:])
            b2sb = spool.tile([N, B], F32)
            nc.scalar.dma_start(out=b2sb[:], in_=b2d[:])

            # ---- stage 1, column-tiled: A = psum rows 0:32 (out1 cols
            # 0:512), B = psum rows 32:64 (out1 cols 512:1024).
            psAB = pool1.tile([2 * B, HB], F32, space="PSUM", tag="ps_ab")

            def mm1(grp, lhsT, rhs, start, stop):
                nc.tensor.matmul(
                    out=psAB[grp * B:(grp + 1) * B, :],
                    lhsT=lhsT, rhs=rhs,
                    start=start, stop=stop, skip_group_check=True,
                )

            mm1(0, ones1[:], b1sb[:1, 0:HB], True, False)
            mm1(1, ones1[:], b1sb[:1, HB:H], True, False)

            # transpose the gather per span-offset chunk, in the order the
            # matmuls consume them: xT[k, n*32+b] = xg[b, n*128+k]
            xtp = poolX.tile([P, P], F32, space="PSUM", tag="xtp")
            xT = spool.tile([P, P], F32)
            for n in (2, 0, 3, 1):
                nc.tensor.transpose(
                    out=xtp[:, n * B:(n + 1) * B],
                    in_=xg[:, n * HS:(n + 1) * HS],
                    identity=ident[:B, :B],
                )
                nc.vector.tensor_copy(
                    out=xT[:, n * B:(n + 1) * B],
                    in_=xtp[:, n * B:(n + 1) * B],
                )

            # matmuls ordered by expected piece arrival (scalar ring first)
            for n, grp, stop in (
                (2, 0, False), (2, 1, False), (0, 0, False), (0, 1, False),
                (3, 0, False), (3, 1, False), (1, 0, True), (1, 1, True),
            ):
                mm1(
                    grp,
                    xT[:, n * B:(n + 1) * B],
                    w1sb[n][:, grp * HB:(grp + 1) * HB],
                    False, stop,
                )

            # ---- stage 2: y[t, b] = b2[t] + sum_o W2[t, o] out1[b, o]
            o1sb = spool.tile([2 * B, HB], F32)
            nc.vector.tensor_copy(out=o1sb[0:B, :], in_=psAB[0:B, :])
            nc.scalar.copy(out=o1sb[B:2 * B, :], in_=psAB[B:2 * B, :])

            trp = poolT.tile([P, 2 * P], F32, space="PSUM", tag="trp")
            trsb = spool.tile([P, 2 * P], F32)
            yps = poolY.tile([N, B], F32, space="PSUM", tag="yps")
            NC = H // P  # 8 chunks of the hidden dim; 0-3 from A, 4-7 from B
            # all transposes back-to-back
            for c in (0, 1, 2, 3, 4, 5, 6, 7):
                g = c // 4
                nc.tensor.transpose(
                    out=trp[:, c * B:(c + 1) * B],
                    in_=o1sb[g * B:(g + 1) * B, (c % 4) * P:(c % 4 + 1) * P],
                    identity=ident[g * B:(g + 1) * B, g * B:(g + 1) * B],
                )
            nc.vector.tensor_copy(out=trsb[:], in_=trp[:])
            for c in range(NC):
                nc.tensor.matmul(
                    out=yps[:],
                    lhsT=w2sb[:, c * N:(c + 1) * N],
                    rhs=trsb[:, c * B:(c + 1) * B],
                    start=(c == 0), stop=(c == NC - 1),
                )

            ysb = spool.tile([N, B], F32)
            nc.vector.tensor_tensor(
                out=ysb[:], in0=yps[:], in1=b2sb[:], op=mybir.AluOpType.add
            )
            nc.sync.dma_start(out=outd[:], in_=ysb[:])

    nc.finalize()
    return nc


def _get_nc():
    global _nc_cache
    if _nc_cache is None:
        _nc_cache = _build_nc()
    return _nc_cache


def kernel(hidden, pos, W1, b1, W2, b2):
    global LAST_EXEC_NS
    hidden = np.asarray(hidden, dtype=np.float32)
    pos = np.asarray(pos)
    W1 = np.asarray(W1, dtype=np.float32)
    b1 = np.asarray(b1, dtype=np.float32)
    W2 = np.asarray(W2, dtype=np.float32)
    b2 = np.asarray(b2, dtype=np.float32)

    # gather row index per batch in the per-core (B*L, HS) layout;
    # the N=4 span rows are contiguous: idxd[b] = b*L + pos[b]
    idxd = (
        np.arange(B, dtype=np.int64) * L + pos.reshape(B).astype(np.int64)
    ).astype(np.int32)[None, :]

    # W1 (H, N*H) -> per-core (N*P, H): w1t_j[n*P+k, o] = W1[o, n*H+j*HS+k]
    w1r = W1.reshape(H, N, NCORES, HS)                 # [o, n, j, k]
    # W2 (N, H) -> (128, 32): w2d[k2, c*N+t] = W2[t, c*P+k2]
    w2d = np.ascontiguousarray(
        W2.reshape(N, H // P, P).transpose(2, 1, 0).reshape(P, -1)
    )
    b1r = b1[None, :]                                  # (1, H), core 0 only
    b1z = np.zeros((1, H), np.float32)
    b2r = np.ascontiguousarray(
        np.broadcast_to(b2[:, None], (N, B))
    )                                                  # (4, 32), core 0 only
    b2z = np.zeros((N, B), np.float32)

    in_maps = []
    for j in range(NCORES):
        hid_j = np.ascontiguousarray(
            hidden[:, :, j * HS:(j + 1) * HS]
        ).reshape(B * L, HS)
        w1t_j = np.ascontiguousarray(
            w1r[:, :, j, :].transpose(1, 2, 0).reshape(N * P, H)
        )
        in_maps.append(
            {
                "hid": hid_j,
                "idxd": idxd,
                "w1t": w1t_j,
                "w2d": w2d,
                "b1d": b1r if j == 0 else b1z,
                "b2d": b2r if j == 0 else b2z,
            }
        )

    nc = _get_nc()
    res = run_bass_kernel_spmd(nc, in_maps, list(range(NCORES)), trace=TRACE)
    LAST_EXEC_NS = res.exec_time_ns

    parts = np.stack([res.results[j]["out"] for j in range(NCORES)])  # (8,4,32)
    ytb = parts.sum(axis=0, dtype=np.float64)                         # (4, 32)
    return np.ascontiguousarray(ytb.T.astype(np.float32))             # (B, N)


# revision 11
# speedup vs baseline: 1.2219x; 1.0669x over previous
"""Trainium2 Bass kernel for nn_Expert (gather-span + 2-layer linear MLP).

Reference computation (B=32, L=4096, H=1024, N=4):
    idx      = pos + arange(N)                      # (B, N)
    gathered = hidden[b, idx[b, n], :]              # (B, N, H)
    x        = gathered.reshape(B, N*H)             # (B, 4096)
    out      = (x @ W1.T + b1) @ W2.T + b2          # (B, 4)

Sharding (8 cores): the contraction dim (N*H = 4096) is split across
cores -- hidden sharded on its last dim in 128-wide slices, W1 sharded
over the matching contraction columns.  Each core reads 2 MB of W1 (the
dominant, irreducible HBM traffic; the problem is memory-bound) plus a
64 KB gathered slice of hidden, computes partial out1/(N,B) y partials,
and the host sums the 8 partials.

Schedule notes (from profiling on this part):
  * The indirect (SWDGE) gather's packets only run when the HWDGE
    rings are idle -- they do NOT round-robin with an active W1
    stream.  So the gather goes FIRST: its descriptors are generated
    while only tiny transfers are in flight, and the sync-ring W1
    pieces are released only after descriptor generation (enforced by
    a WAW dep: tiny gpsimd memsets into the W1 tiles, ordered after
    the indirect DMA).  The scalar ring streams immediately -- the
    gather slots in after its first 256 KB batch.
  * The gather is 32x 2KB descriptors: one per batch row, exploiting
    that the N=4 span rows are contiguous in (B*L, 128) layout.
  * fp32 matmuls run LOW+HIGH passes at the HAM-cold 1.2 GHz rate
    (~430ns per 512-col pass); stage 1 is column-tiled (group A ->
    psum partitions 0:32 = out1[:, 0:512], group B -> 32:64, col_grp
    q32) so the two groups' passes run concurrently and PE stays
    under the DMA rate.
  * b1 enters PSUM via K=1 matmuls (group start) while the PE is
    otherwise idle; b2 rides a (4,32) pre-broadcast add at the end.
  * Stage 2 is straight-line batches (no PE<->DVE ping-pong): copy
    out1 to SBUF (vector || scalar), 8 PE transposes alternating row
    strips q0/q32, one copy, 8 accumulating y matmuls.
The host sums per-core y partials (fp64) and transposes to (B, N).
"""

import numpy as np

from concourse import bass, bacc, mybir
from concourse.tile import TileContext
from concourse.bass_utils import run_bass_kernel_spmd
from concourse.masks import make_identity

B, L, H, N = 32, 4096, 1024, 4
NCORES = 8
HS = H // NCORES       # 128: per-core slice of the hidden dim
P = 128
HB = H // 2            # 512: psum bank width for stage 1
F32 = mybir.dt.float32
I32 = mybir.dt.int32

TRACE = False          # set True in test harnesses to profile
LAST_EXEC_NS = None

_nc_cache = None


def _build_nc():
    nc = bacc.Bacc(target_bir_lowering=False)
    hid = nc.declare_dram_parameter("hid", [B * L, HS], F32, isOutput=False)
    idxd = nc.declare_dram_parameter("idxd", [1, B], I32, isOutput=False)
    w1t = nc.declare_dram_parameter("w1t", [N * P, H], F32, isOutput=False)
    w2d = nc.declare_dram_parameter("w2d", [P, 2 * N * N], F32, isOutput=False)
    b1d = nc.declare_dram_parameter("b1d", [1, H], F32, isOutput=False)
    b2d = nc.declare_dram_parameter("b2d", [N, B], F32, isOutput=False)
    outd = nc.declare_dram_parameter("out", [N, B], F32, isOutput=True)

    with TileContext(nc) as tc:
        with (
            tc.tile_pool(name="sbuf", bufs=1) as spool,
            tc.tile_pool(name="ps1", bufs=1, space="PSUM") as pool1,
            tc.tile_pool(name="psX", bufs=1, space="PSUM") as poolX,
            tc.tile_pool(name="psT", bufs=1, space="PSUM") as poolT,
            tc.tile_pool(name="psY", bufs=1, space="PSUM") as poolY,
        ):
            w1sb = [
                spool.tile([P, H], F32, tag=f"w1_{n}", name=f"w1_{n}")
                for n in range(N)
            ]

            def w1piece(n, h):
                eng = nc.sync if n in (0, 1) else nc.scalar
                eng.dma_start(
                    out=w1sb[n][:, h * HB:(h + 1) * HB],
                    in_=w1t[n * P:(n + 1) * P, h * HB:(h + 1) * HB],
                )

            # sync ring: gather indices first
            idxi = spool.tile([B, 1], I32)
            nc.sync.dma_start(out=idxi[:], in_=idxd[:])
            # scalar ring: b1 first (unlocks the psum-init matmuls)
            b1sb = spool.tile([1, H], F32)
            nc.scalar.dma_start(out=b1sb[:], in_=b1d[:])

            # ---- identity for PE transposes; ones row for the b1 matmul
            ident = spool.tile([P, P], F32)
            make_identity(nc, ident[:])
            ones1 = spool.tile([1, B], F32)
            nc.vector.memset(ones1[:], 1.0)

            # ---- gather: 32 indices, 4 contiguous 512B rows per index
            xg = spool.tile([B, N * HS], F32)
            nc.gpsimd.indirect_dma_start(
                out=xg[:, :],
                out_offset=None,
                in_=hid[:],
                in_offset=bass.IndirectOffsetOnAxis(ap=idxi[:, :1], axis=0),
                bounds_check=B * L - 1,
                oob_is_err=False,
            )
            # WAW release: the sync-ring W1 tile DMAs below overwrite these
            # rows, so they must wait for the memsets -- which sit after the
            # indirect_dma on the gpsimd queue.  Net effect: sync-ring W1
            # traffic starts only once the gather's descriptors are in the
            # rings (SWDGE packets starve under an active HWDGE stream).
            GATE_W1 = True
            if GATE_W1:
                nc.gpsimd.memset(w1sb[0][0:1, :], 0.0)
                nc.gpsimd.memset(w1sb[1][0:1, :], 0.0)

            # W1 pieces: scalar ring (tiles 2/3) streams immediately; sync
            # ring (tiles 0/1) is gated per the WAW above.
            for h in range(2):
                w1piece(0, h)
                w1piece(2, h)
            for h in range(2):
                w1piece(1, h)
                w1piece(3, h)
            w2sb = spool.tile([P, 2 * N * N], F32)
            nc.scalar.dma_start(out=w2sb[:], in_=w2d[:])
            b2sb = spool.tile([N, B], F32)
            nc.scalar.dma_start(out=b2sb[:], in_=b2d[:])

            # ---- stage 1, column-tiled: A = psum rows 0:32 (out1 cols
            # 0:512), B = psum rows 32:64 (out1 cols 512:1024).
            psAB = pool1.tile([2 * B, HB], F32, space="PSUM", tag="ps_ab")

            def mm1(grp, lhsT, rhs, start, stop):
                nc.tensor.matmul(
                    out=psAB[grp * B:(grp + 1) * B, :],
                    lhsT=lhsT, rhs=rhs,
                    start=start, stop=stop, skip_group_check=True,
                )

            mm1(0, ones1[:], b1sb[:1, 0:HB], True, False)
            mm1(1, ones1[:], b1sb[:1, HB:H], True, False)

            # transpose the gather per span-offset chunk, in the order the
            # matmuls consume them: xT[k, n*32+b] = xg[b, n*128+k]
            xtp = poolX.tile([P, P], F32, space="PSUM", tag="xtp")
            xT = spool.tile([P, P], F32)
            for n in (2, 0, 3, 1):
                nc.tensor.transpose(
                    out=xtp[:, n * B:(n + 1) * B],
                    in_=xg[:, n * HS:(n + 1) * HS],
                    identity=ident[:B, :B],
                )
                nc.vector.tensor_copy(
                    out=xT[:, n * B:(n + 1) * B],
                    in_=xtp[:, n * B:(n + 1) * B],
                )

            # matmuls ordered by expected piece arrival (scalar ring first)
            for n, grp, stop in (
                (2, 0, False), (2, 1, False), (0, 0, False), (0, 1, False),
                (3, 0, False), (3, 1, False), (1, 0, True), (1, 1, True),
            ):
                mm1(
                    grp,
                    xT[:, n * B:(n + 1) * B],
                    w1sb[n][:, grp * HB:(grp + 1) * HB],
                    False, stop,
                )

            # ---- stage 2: y[t, b] = b2[t] + sum_o W2[t, o] out1[b, o]
            o1sb = spool.tile([2 * B, HB], F32)
            nc.vector.tensor_copy(out=o1sb[0:B, :], in_=psAB[0:B, :])
            nc.scalar.copy(out=o1sb[B:2 * B, :], in_=psAB[B:2 * B, :])

            trp = poolT.tile([P, 2 * P], F32, space="PSUM", tag="trp")
            trsb = spool.tile([P, 2 * P], F32)
            yps = poolY.tile([N, B], F32, space="PSUM", tag="yps")
            NC = H // P  # 8 chunks of the hidden dim; 0-3 from A, 4-7 from B
            # all transposes back-to-back
            for c in (0, 1, 2, 3, 4, 5, 6, 7):
                g = c // 4
                nc.tensor.transpose(
                    out=trp[:, c * B:(c + 1) * B],
                    in_=o1sb[g * B:(g + 1) * B, (c % 4) * P:(c % 4 + 1) * P],
                    identity=ident[g * B:(g + 1) * B, g * B:(g + 1) * B],
                )
            nc.vector.tensor_copy(out=trsb[:], in_=trp[:])
            for c in range(NC):
                nc.tensor.matmul(
                    out=yps[:],
                    lhsT=w2sb[:, c * N:(c + 1) * N],
                    rhs=trsb[:, c * B:(c + 1) * B],
                    start=(c == 0), stop=(c == NC - 1),
                )

            ysb = spool.tile([N, B], F32)
            nc.vector.tensor_tensor(
                out=ysb[:], in0=yps[:], in1=b2sb[:], op=mybir.AluOpType.add
            )
            nc.sync.dma_start(out=outd[:], in_=ysb[:])

    nc.finalize()
    return nc


def _get_nc():
    global _nc_cache
    if _nc_cache is None:
        _nc_cache = _build_nc()
    return _nc_cache


def kernel(hidden, pos, W1, b1, W2, b2):
    global LAST_EXEC_NS
    hidden = np.asarray(hidden, dtype=np.float32)
    pos = np.asarray(pos)
    W1 = np.asarray(W1, dtype=np.float32)
    b1 = np.asarray(b1, dtype=np.float32)
    W2 = np.asarray(W2, dtype=np.float32)
    b2 = np.asarray(b2, dtype=np.float32)

    # gather row index per batch in the per-core (B*L, HS) layout;
    # the N=4 span rows are contiguous: idxd[b] = b*L + pos[b]
    idxd = (
        np.arange(B, dtype=np.int64) * L + pos.reshape(B).astype(np.int64)
    ).astype(np.int32)[None, :]

    # W1 (H, N*H) -> per-core (N*P, H): w1t_j[n*P+k, o] = W1[o, n*H+j*HS+k]
    w1r = W1.reshape(H, N, NCORES, HS)                 # [o, n, j, k]
    # W2 (N, H) -> (128, 32): w2d[k2, c*N+t] = W2[t, c*P+k2]
    w2d = np.ascontiguousarray(
        W2.reshape(N, H // P, P).transpose(2, 1, 0).reshape(P, -1)
    )
    b1r = b1[None, :]                                  # (1, H), core 0 only
    b1z = np.zeros((1, H), np.float32)
    b2r = np.ascontiguousarray(
        np.broadcast_to(b2[:, None], (N, B))
    )                                                  # (4, 32), core 0 only
    b2z = np.zeros((N, B), np.float32)

    in_maps = []
    for j in range(NCORES):
        hid_j = np.ascontiguousarray(
            hidden[:, :, j * HS:(j + 1) * HS]
        ).reshape(B * L, HS)
        w1t_j = np.ascontiguousarray(
            w1r[:, :, j, :].transpose(1, 2, 0).reshape(N * P, H)
        )
        in_maps.append(
            {
                "hid": hid_j,
                "idxd": idxd,
                "w1t": w1t_j,
                "w2d": w2d,
                "b1d": b1r if j == 0 else b1z,
                "b2d": b2r if j == 0 else b2z,
            }
        )

    nc = _get_nc()
    res = run_bass_kernel_spmd(nc, in_maps, list(range(NCORES)), trace=TRACE)
    LAST_EXEC_NS = res.exec_time_ns

    parts = np.stack([res.results[j]["out"] for j in range(NCORES)])  # (8,4,32)
    ytb = parts.sum(axis=0, dtype=np.float64)                         # (4, 32)
    return np.ascontiguousarray(ytb.T.astype(np.float32))             # (B, N)
